# revision 10
# baseline (speedup 1.0000x reference)
"""Trainium2 Bass kernel for nn_Normalizer (annealed top-k masking normalizer).

Math (see reference): the T=20 annealed-theta loop converges; the output
depends only on the fixed point c* of  s(c) = k*c  where, in exp space,
E = exp(masked_score/theta),  s(c) = sum_j min(E_j, c),  k = 0.1 * n_finite.
The scheduled trajectory's c_19 differs from c* by ~1e-4 relative, far
below the accuracy gate, so the kernel solves the fixed point directly:

  1. host: sm = fp16(score, masked -> -60000)           [halves input DMA]
  2. ACT:  E = bf16(exp(sm/theta)) per 128-row tile, in column halves
     (quarters for tile 0) so compute starts as soon as DMA lands
  3. k = 0.1 * count(finite): DVE slice counts sm > -1000 (during the exp
     phase, off the fp16 input); ACT slice counts via
     sum(relu(1 - HUGE*E)) = width - count (exact: masked E is exactly 0);
     emitted after the sub phase so they fill ACT gaps -- only needed by
     the first full-width update.
  4. sub phase on DVE: 5 fixed-point iters on a 1/16 column subsample
     (8 cols every 128) via a strided view of E; tiles {0,1} start during
     the exp phase, {2,3} follow; updates batched [128,2] (group B's on
     the otherwise-idle Pool engine).
  5. full phase "FSS" as TWO independent chains (tiles {0,1} and {2,3}):
     three full-width s(c) passes each, column-sliced across DVE
     (min+accum) and ACT (relu-trick: sum min = W*c - sum relu(c-E));
     update 1 = plain fixed point c=s/k, updates 2,3 = secant from the
     last two (c, s) pairs -- no full count passes.  The two chains
     interleave on the engines, hiding each other's update latency, and
     the c-only secant inputs (dc, dc2, rdc, kc) are computed during the
     s-pass on Pool.
  6. per group: gamma = bf16(min(E * (1/c), 1)) in place over E (DVE 4x
     mode, halves), DMA out as bf16 as soon as that group's c is final;
     host upcasts to f32.

The Pool engine only supports tensor_tensor add/mult/sub + memset in this
toolchain, so it gets exactly those.

Sharding: pure row-parallel, 4096 rows -> 8 cores x 512 rows.
"""

import os
import sys

import numpy as np

try:
    import concourse.bass as bass  # noqa: F401
except ImportError:
    sys.path.insert(0, "/opt/trn_rl_repo")
    import concourse.bass as bass  # noqa: F401

import ml_dtypes  # noqa: F401

import concourse.bacc as bacc
import concourse.tile as tile
from concourse import mybir
from concourse.bass_utils import run_bass_kernel_spmd

F32 = mybir.dt.float32
BF16 = mybir.dt.bfloat16
FP16 = mybir.dt.float16
A = mybir.AluOpType
AF = mybir.ActivationFunctionType

THETA, P_FRAC = 0.3, 0.1
BSZ, SEQ = 4096, 8192
N_CORES = 8
ROWS_PER_CORE = BSZ // N_CORES          # 512
P = 128
N_TILES = ROWS_PER_CORE // P            # 4
HALF = SEQ // 2
CHUNK = int(os.environ.get("NORM_SUB_CHUNK", "8"))
CHUNK_EVERY = 64                        # within the first half
SUB = HALF // CHUNK_EVERY * CHUNK
BIG = 1.0e30
HH = 1.0e25                             # relu count scaling
MASKVAL = -60000.0                      # fp16-representable, exp -> 0
SM_THRESH = -1000.0                     # finite iff sm > this

N_SUB = int(os.environ.get("NORM_SUB_ITERS", "5"))
FULL_SEQ = os.environ.get("NORM_FULL_SEQ", "FSS")  # F=fixed point, S=secant
# column-slice widths (DVE vs ACT) for the full s-passes and k-passes
S_DVE = int(os.environ.get("NORM_S_DVE", "4096"))
S_ACT = SEQ - S_DVE
K_DVE = int(os.environ.get("NORM_K_DVE", "4608"))
K_ACT = SEQ - K_DVE
DEBUG = os.environ.get("NORM_DEBUG", "0") == "1"


def _sub_view(ap):
    """[P, SEQ] AP -> [P, 64, CHUNK] strided subsample view of the first
    half (columns are iid, so sampling only the first half is fine and
    lets the sub phase start before the second half-DMA lands)."""
    return ap[:, 0:HALF].rearrange(
        "p (c l) -> p c l", l=CHUNK_EVERY)[:, :, 0:CHUNK]


def _sub_out(ap):
    """[P, SUB] contiguous AP -> [P, 64, CHUNK] view."""
    return ap.rearrange("p (c l) -> p c l", l=CHUNK)


def build_kernel():
    nc = bacc.Bacc("TRN2", target_bir_lowering=False, debug=False,
                   num_devices=N_CORES)
    sm_d = nc.dram_tensor("sm", [ROWS_PER_CORE, SEQ], FP16,
                          kind="ExternalInput")
    gamma_d = nc.dram_tensor("gamma", [ROWS_PER_CORE, SEQ], BF16,
                             kind="ExternalOutput")
    dbg_d = nc.dram_tensor("dbg", [P, 64], F32,
                           kind="ExternalOutput") if DEBUG else None
    dbg_tiles = []

    def dbg(name, t, w=N_TILES):
        if DEBUG:
            dbg_tiles.append((name, t, w))

    v = nc.vector
    g = nc.gpsimd
    s = nc.scalar

    with tile.TileContext(nc) as tc:
        with (
            tc.tile_pool(name="smp", bufs=1) as smp,
            tc.tile_pool(name="ep", bufs=1) as ep,
            tc.tile_pool(name="jdp", bufs=1) as jdp,
            tc.tile_pool(name="jap", bufs=1) as jap,
            tc.tile_pool(name="jsp", bufs=1) as jsp,
            tc.tile_pool(name="scal", bufs=8) as scal,
        ):
            jD = jdp.tile([P, max(K_DVE, S_DVE, SUB)], F32, tag="jD")
            jA = jap.tile([P, max(K_ACT, S_ACT)], F32, tag="jA")
            jSD = jsp.tile([P, SUB], F32, tag="jSD")
            jSA = jsp.tile([P, SUB], F32, tag="jSA")

            cnts4 = scal.tile([P, N_TILES], F32, tag="cnts")
            kD4 = scal.tile([P, N_TILES], F32, tag="kD")
            rkA4 = scal.tile([P, N_TILES], F32, tag="rkA")
            cSUB = scal.tile([P, 2], F32, tag="cSUB")
            g.memset(cSUB[:], float(SUB))
            eps30 = scal.tile([P, 2], F32, tag="eps30")
            g.memset(eps30[:], 1e-30)

            # ---- phase A: DMA in, exp, DVE count slices ----------------
            E = []
            for j in range(N_TILES):
                r0 = j * P
                sm = smp.tile([P, SEQ], FP16, tag=f"sm{j % 2}")
                splits = 2
                w = SEQ // splits
                for q in range(splits):
                    nc.sync.dma_start(out=sm[:][:, q * w:(q + 1) * w],
                                      in_=sm_d.ap()[r0:r0 + P,
                                                    q * w:(q + 1) * w])
                e_t = ep.tile([P, SEQ], BF16, tag=f"E{j}")
                E.append(e_t)
                for q in range(splits):
                    s.activation(out=e_t[:][:, q * w:(q + 1) * w],
                                 in_=sm[:][:, q * w:(q + 1) * w],
                                 func=AF.Exp, scale=1.0 / THETA)
                # full-count DVE slice off sm
                v.tensor_scalar(out=jD[:][:, 0:K_DVE],
                                in0=sm[:][:, 0:K_DVE],
                                scalar1=SM_THRESH, scalar2=None,
                                op0=A.is_gt, op1=A.add,
                                accum_out=kD4[:, j:j + 1])
                # subsample count off sm
                v.tensor_scalar(out=_sub_out(jSD[:]), in0=_sub_view(sm[:]),
                                scalar1=SM_THRESH, scalar2=None,
                                op0=A.is_gt, op1=A.add,
                                accum_out=cnts4[:, j:j + 1])

            # per-group subsample rks = 10 / cnt_sub
            rks = []
            for grp in range(2):
                rc_ = scal.tile([P, 2], F32, tag=f"rcs{grp}")
                v.reciprocal(rc_[:], cnts4[:, 2 * grp:2 * grp + 2])
                rk_ = scal.tile([P, 2], F32, tag=f"rks{grp}")
                v.tensor_scalar_mul(rk_[:], rc_[:], 1.0 / P_FRAC)
                rks.append(rk_)

            # ---- phase B: subsample fixed point, 2 chains ---------------
            # group A (tiles 0,1) on DVE; group B (tiles 2,3) on ACT with
            # updates on Pool
            cg = [None, None]
            for it in range(N_SUB):
                sA = scal.tile([P, 2], F32, tag="sg0")
                for jj in range(2):
                    v.tensor_scalar(out=_sub_out(jSD[:]),
                                    in0=_sub_view(E[jj][:]),
                                    scalar1=(BIG if it == 0 else
                                             cg[0][:, jj:jj + 1]),
                                    scalar2=None,
                                    op0=A.min, op1=A.add,
                                    accum_out=sA[:, jj:jj + 1])
                cn_ = scal.tile([P, 2], F32, tag="cg0")
                v.tensor_mul(cn_[:], sA[:], rks[0][:])
                cg[0] = cn_
            for it in range(N_SUB):
                rB = scal.tile([P, 2], F32, tag="sg1")
                for jj in range(2):
                    if it == 0:
                        s.activation(out=_sub_out(jSA[:]),
                                     in_=_sub_view(E[2 + jj][:]),
                                     func=AF.Identity,
                                     accum_out=rB[:, jj:jj + 1])
                    else:
                        s.activation(out=_sub_out(jSA[:]),
                                     in_=_sub_view(E[2 + jj][:]),
                                     func=AF.Relu, scale=-1.0,
                                     bias=cg[1][:, jj:jj + 1],
                                     accum_out=rB[:, jj:jj + 1])
                cn_ = scal.tile([P, 2], F32, tag="cg1")
                if it == 0:
                    g.tensor_mul(cn_[:], rB[:], rks[1][:])
                else:
                    uB = scal.tile([P, 2], F32, tag="ug1")
                    g.tensor_mul(uB[:], cg[1][:], cSUB[:])
                    tB = scal.tile([P, 2], F32, tag="tg1")
                    g.tensor_sub(tB[:], uB[:], rB[:])
                    g.tensor_mul(cn_[:], tB[:], rks[1][:])
                cg[1] = cn_

            # ---- ACT count slices (fill ACT gaps) + k prep --------------
            for j in range(N_TILES):
                s.activation(out=jA[:][:, 0:K_ACT],
                             in_=E[j][:][:, K_DVE:SEQ],
                             func=AF.Relu, scale=-HH, bias=1.0,
                             accum_out=rkA4[:, j:j + 1])
            kg, rkg, k02g = [], [], []
            for grp in range(2):
                gsl = slice(2 * grp, 2 * grp + 2)
                t1 = scal.tile([P, 2], F32, tag=f"t1{grp}")
                v.scalar_tensor_tensor(out=t1[:], in0=rkA4[:, gsl],
                                       scalar=-1.0, in1=kD4[:, gsl],
                                       op0=A.mult, op1=A.add)
                cnt2 = scal.tile([P, 2], F32, tag=f"cnt{grp}")
                v.tensor_scalar_add(cnt2[:], t1[:], float(K_ACT))
                k2 = scal.tile([P, 2], F32, tag=f"k{grp}")
                v.tensor_scalar_mul(k2[:], cnt2[:], P_FRAC)
                rk2 = scal.tile([P, 2], F32, tag=f"rkk{grp}")
                v.reciprocal(rk2[:], k2[:])
                k022 = scal.tile([P, 2], F32, tag=f"k02{grp}")
                v.tensor_scalar_mul(k022[:], k2[:], 0.02)
                kg.append(k2)
                rkg.append(rk2)
                k02g.append(k022)
            dbg("cnts4", cnts4)
            dbg("kD4", kD4)
            dbg("rkA4", rkA4)

            # ---- phase C: full-width passes, two independent chains -----
            def s_pass_group(c2, grp, tag):
                """one full-width s(c) measurement for tiles {2g, 2g+1}."""
                sD2 = scal.tile([P, 2], F32, tag="sD" + tag)
                rA2 = scal.tile([P, 2], F32, tag="rA" + tag)
                for jj in range(2):
                    j = 2 * grp + jj
                    cj = c2[:, jj:jj + 1]
                    v.tensor_scalar(out=jD[:][:, 0:S_DVE],
                                    in0=E[j][:][:, 0:S_DVE],
                                    scalar1=cj, scalar2=None,
                                    op0=A.min, op1=A.add,
                                    accum_out=sD2[:, jj:jj + 1])
                    s.activation(out=jA[:][:, 0:S_ACT],
                                 in_=E[j][:][:, S_DVE:SEQ],
                                 func=AF.Relu, scale=-1.0, bias=cj,
                                 accum_out=rA2[:, jj:jj + 1])
                u1 = scal.tile([P, 2], F32, tag="u1" + tag)
                v.scalar_tensor_tensor(out=u1[:], in0=c2[:],
                                       scalar=float(S_ACT), in1=rA2[:],
                                       op0=A.mult, op1=A.subtract)
                s2 = scal.tile([P, 2], F32, tag="s2" + tag)
                v.tensor_add(s2[:], sD2[:], u1[:])
                return s2

            def gamma_group(grp):
                rc2 = scal.tile([P, 2], F32, tag=f"rc{grp}")
                v.reciprocal(rc2[:], cg[grp][:])
                for jj in range(2):
                    j = 2 * grp + jj
                    r0 = j * P
                    for h0, h1 in ((0, HALF), (HALF, SEQ)):
                        v.tensor_scalar(out=E[j][:][:, h0:h1],
                                        in0=E[j][:][:, h0:h1],
                                        scalar1=rc2[:, jj:jj + 1],
                                        scalar2=1.0,
                                        op0=A.mult, op1=A.min)
                        nc.sync.dma_start(
                            out=gamma_d.ap()[r0:r0 + P, h0:h1],
                            in_=E[j][:][:, h0:h1])

            cps = [None, None]
            sps = [None, None]
            last = len(FULL_SEQ) - 1
            for i, stepc in enumerate(FULL_SEQ):
                for grp in range(2):
                    c2 = cg[grp]
                    if stepc != "F":
                        dc = scal.tile([P, 2], F32, tag=f"dc{i}{grp}")
                        g.tensor_sub(dc[:], c2[:], cps[grp][:])
                        ec = scal.tile([P, 2], F32, tag=f"ec{i}{grp}")
                        g.tensor_mul(ec[:], c2[:], eps30[:])
                        dc2 = scal.tile([P, 2], F32, tag=f"dc2{i}{grp}")
                        g.tensor_add(dc2[:], dc[:], ec[:])
                        kc = scal.tile([P, 2], F32, tag=f"kc{i}{grp}")
                        g.tensor_mul(kc[:], kg[grp][:], c2[:])
                        rdc = scal.tile([P, 2], F32, tag=f"rdc{i}{grp}")
                        v.reciprocal(rdc[:], dc2[:])
                    s2 = s_pass_group(c2, grp, f"f{i}g{grp}")
                    cn = scal.tile([P, 2], F32, tag=f"c{i}g{grp}")
                    if stepc == "F":
                        v.tensor_mul(cn[:], s2[:], rkg[grp][:])
                    else:
                        ds = scal.tile([P, 2], F32, tag=f"ds{i}{grp}")
                        v.tensor_sub(ds[:], s2[:], sps[grp][:])
                        m_ = scal.tile([P, 2], F32, tag=f"m{i}{grp}")
                        v.tensor_mul(m_[:], ds[:], rdc[:])
                        den = scal.tile([P, 2], F32, tag=f"den{i}{grp}")
                        v.tensor_sub(den[:], kg[grp][:], m_[:])
                        den2 = scal.tile([P, 2], F32, tag=f"den2{i}{grp}")
                        v.tensor_max(den2[:], den[:], k02g[grp][:])
                        rden = scal.tile([P, 2], F32, tag=f"rden{i}{grp}")
                        v.reciprocal(rden[:], den2[:])
                        num = scal.tile([P, 2], F32, tag=f"num{i}{grp}")
                        v.tensor_sub(num[:], s2[:], kc[:])
                        tq = scal.tile([P, 2], F32, tag=f"tq{i}{grp}")
                        v.tensor_mul(tq[:], num[:], rden[:])
                        v.tensor_add(cn[:], c2[:], tq[:])
                    cps[grp], sps[grp] = c2, s2
                    cg[grp] = cn
                    if i == last and grp == 0:
                        # group A's gamma + output DMA overlap group B's
                        # final pass
                        gamma_group(0)
            gamma_group(1)

            if DEBUG:
                off = 0
                for name, t, w in dbg_tiles:
                    nc.sync.dma_start(out=dbg_d.ap()[:, off:off + w],
                                      in_=t[:])
                    off += w

    nc.compile()
    return nc


_NC_CACHE = None


def prep_sm(score: np.ndarray, mask: np.ndarray) -> np.ndarray:
    """host-side dtype prep: masked score in fp16 (elementwise only)."""
    return np.where(np.asarray(mask) == 0, np.float16(MASKVAL),
                    np.asarray(score).astype(np.float16))


def kernel(score: np.ndarray, mask: np.ndarray) -> np.ndarray:
    global _NC_CACHE
    if _NC_CACHE is None:
        _NC_CACHE = build_kernel()
    nc = _NC_CACHE

    sm16 = np.ascontiguousarray(prep_sm(score, mask))
    in_maps = []
    for i in range(N_CORES):
        sl = slice(i * ROWS_PER_CORE, (i + 1) * ROWS_PER_CORE)
        in_maps.append({"sm": sm16[sl]})
    res = run_bass_kernel_spmd(nc, in_maps, core_ids=list(range(N_CORES)))
    out = np.concatenate([res.results[i]["gamma"] for i in range(N_CORES)],
                         axis=0)
    return out.astype(np.float32)
